# revision 12
# baseline (speedup 1.0000x reference)
"""CropSampler Trainium2 kernel.

Strategy (data-parallel over 8 cores, 2 images per core):
  - Bilinear crop-resample is separable:  out = Wv^T @ C @ Wh  where C is the
    crop window of one channel and Wv/Wh are per-axis 2-tap weight matrices
    (masks/clip semantics folded in on host, incl. the reference's x1_in=x0_in
    bug and clipped-weight degenerate zeros).
  - Host re-pitches each image channel to a uniform 1920-float row stride
    (pure layout change) so crop loads are static-shape strided DMAs whose
    base offset comes from a per-core metadata tensor via a register
    (SPMD-uniform program; all per-core variation is data).
  - Device: HWDGE DMA crop rows -> PE Pass1 (vertical, accumulate over row
    chunks) -> PSUM -> copy -> PE Pass2 (horizontal) -> PSUM -> copy -> DMA out.
    flat coords are generated on device by a K=2 matmul (affine expansion).
"""
import sys
sys.path.insert(0, "/opt/trn_rl_repo")

import numpy as np

import concourse.bass as bass
import concourse.bacc as bacc
import concourse.mybir as mybir
import concourse.tile as tile
from concourse import bass_utils
from concourse.bass import ds

CROP = 256
PITCH = 1920
NCORES = 8
RUN_KWARGS = {}
F32 = mybir.dt.float32
I32 = mybir.dt.int32


# ---------------------------------------------------------------- host geometry
def image_geometry(center_b, bbox_b, h, w):
    s = np.float32(bbox_b)
    tx = np.float32(np.float32(center_b[0]) - np.float32(s * np.float32(0.5)))
    ty = np.float32(np.float32(center_b[1]) - np.float32(s * np.float32(0.5)))
    lin = (np.arange(CROP, dtype=np.float32) / np.float32(CROP - 1)).astype(np.float32)
    xs = (lin * s + tx).astype(np.float32)
    ys = (lin * s + ty).astype(np.float32)
    x0 = np.floor(xs).astype(np.int64)
    y0 = np.floor(ys).astype(np.int64)
    vx = (x0 >= 0) & (x0 <= w - 2)
    vy = (y0 >= 0) & (y0 <= h - 2)
    xlo = int(x0[vx].min()) if vx.any() else 0
    ylo = int(y0[vy].min()) if vy.any() else 0
    cw = (int(x0[vx].max()) + 1 - xlo + 1) if vx.any() else 2
    ch = (int(y0[vy].max()) + 1 - ylo + 1) if vy.any() else 2
    return dict(xs=xs, ys=ys, x0=x0, y0=y0, vx=vx, vy=vy,
                xlo=xlo, ylo=ylo, cw=cw, ch=ch, h=h, w=w,
                s=s, tx=tx, ty=ty)


def build_w(vals, v0, valid, lo, n_rows):
    """2-tap weight matrix [n_rows, 256]: row (v0-lo) gets (v0+1-val), row+1 gets (val-v0)."""
    W = np.zeros((n_rows, CROP), np.float32)
    for j in range(CROP):
        if not valid[j]:
            continue
        k = int(v0[j]) - lo
        W[k, j] += np.float32(np.float32(v0[j] + 1) - vals[j])
        W[k + 1, j] += np.float32(vals[j] - np.float32(v0[j]))
    return W


# ---------------------------------------------------------------- device program
def build_program(R, L, Hs):
    """R[s]: padded crop rows (mult 128), L[s]: padded crop cols (mult 128),
    Hs[s]: channel-buffer heights. Returns (nc, names)."""
    nc = bacc.Bacc("TRN2", target_bir_lowering=False, debug=False,
                   enable_asserts=False, num_devices=NCORES)
    T = [R[s] // 128 for s in range(2)]
    M = [L[s] // 128 for s in range(2)]

    img = [nc.dram_tensor(f"img{s}", (Hs[s],), F32,
                          kind="ExternalInput").ap() for s in range(2)]
    wv = [nc.dram_tensor(f"wv{s}", (R[s], CROP), F32, kind="ExternalInput").ap()
          for s in range(2)]
    wh = [nc.dram_tensor(f"wh{s}", (L[s], CROP), F32, kind="ExternalInput").ap()
          for s in range(2)]
    meta = nc.dram_tensor("meta", (128, 2 * 3 * max(T)), I32, kind="ExternalInput").ap()
    frhs = nc.dram_tensor("frhs", (2, 2 * 512), F32, kind="ExternalInput").ap()
    flhs = nc.dram_tensor("flhs", (2, 2 * 128), F32, kind="ExternalInput").ap()
    tfhd = nc.dram_tensor("tfhd", (1, 36), F32, kind="ExternalInput").ap()

    images_out = nc.dram_tensor("images_out", (2, 3, CROP, CROP), F32,
                                kind="ExternalOutput").ap()
    flat_out = nc.dram_tensor("flat_out", (2, CROP * CROP, 2), F32,
                              kind="ExternalOutput").ap()
    tfhd_out = nc.dram_tensor("tfhd_out", (1, 36), F32, kind="ExternalOutput").ap()

    with tile.TileContext(nc) as tc:
        with (
            tc.tile_pool(name="wpool", bufs=1) as wpool,
            tc.tile_pool(name="sb", bufs=6) as sb,
            tc.tile_pool(name="t1sb", bufs=2) as t1pool,
            tc.tile_pool(name="outp", bufs=2) as outpool,
            tc.tile_pool(name="ps", bufs=1, space="PSUM") as ps,
            tc.tile_pool(name="psо", bufs=2, space="PSUM") as pso,
        ):
            # --- persistent weight tiles ---
            wv_sb, wh_sb = [], []
            for s in range(2):
                wt = wpool.tile([128, T[s] * CROP], F32, tag=f"wv{s}")
                nc.sync.dma_start(
                    out=wt[:].rearrange("p (t i) -> p t i", t=T[s]),
                    in_=wv[s].rearrange("(t p) i -> p t i", p=128))
                wv_sb.append(wt)
                ht = wpool.tile([128, M[s] * CROP], F32, tag=f"wh{s}")
                nc.sync.dma_start(
                    out=ht[:].rearrange("p (m i) -> p m i", m=M[s]),
                    in_=wh[s].rearrange("(m p) i -> p m i", p=128))
                wh_sb.append(ht)
            meta_sb = wpool.tile([128, 2 * 3 * max(T)], I32, tag="meta")
            nc.sync.dma_start(out=meta_sb[:], in_=meta)
            frhs_sb = wpool.tile([2, 2 * 512], F32, tag="frhs")
            nc.sync.dma_start(out=frhs_sb[:], in_=frhs)
            flhs_sb = wpool.tile([2, 2 * 128], F32, tag="flhs")
            nc.sync.dma_start(out=flhs_sb[:], in_=flhs)
            tf_sb = wpool.tile([1, 36], F32, tag="tfhd")
            nc.sync.dma_start(out=tf_sb[:], in_=tfhd)
            nc.sync.dma_start(out=tfhd_out, in_=tf_sb[:])

            # --- flat coords via K=2 matmul ---
            for s in range(2):
                for ci in range(2):
                    fp = pso.tile([128, 512], F32, tag="flatp")
                    nc.tensor.matmul(
                        fp[:],
                        lhsT=flhs_sb[:, ci * 128:(ci + 1) * 128],
                        rhs=frhs_sb[:, s * 512:(s + 1) * 512],
                        start=True, stop=True)
                    fsb = outpool.tile([128, 512], F32, tag="flatsb")
                    nc.scalar.copy(out=fsb[:], in_=fp[:])
                    nc.sync.dma_start(
                        out=flat_out[s].rearrange(
                            "(ci p j) two -> p ci (j two)", ci=2, p=128)[:, ci, :],
                        in_=fsb[:])

            # --- main per (slot, channel) pipeline ---
            for s in range(2):
                n_t1b = (M[s] + 1) // 2
                for c in range(3):
                    crops = []
                    for t in range(T[s]):
                        col = (s * 3 + c) * max(T) + t
                        ct = sb.tile([128, L[s]], F32, tag=f"crop{s}")
                        nc.gpsimd.indirect_dma_start(
                            out=ct[:],
                            out_offset=None,
                            in_=bass.AP(img[s].tensor, 0, [[1, Hs[s]], [1, 1]]),
                            in_offset=bass.IndirectOffsetOnAxis(
                                ap=meta_sb[:, col:col + 1], axis=0))
                        crops.append(ct)
                    t1b = [ps.tile([128, 512], F32, tag=f"t1_{mb}", name=f"t1_{s}_{c}_{mb}")
                           for mb in range(n_t1b)]
                    for m in range(M[s]):
                        for t in range(T[s]):
                            nc.tensor.matmul(
                                t1b[m // 2][:, (m % 2) * 256:(m % 2) * 256 + 256],
                                lhsT=crops[t][:, m * 128:(m + 1) * 128],
                                rhs=wv_sb[s][:, t * 256:(t + 1) * 256],
                                start=(t == 0), stop=(t == T[s] - 1))
                    t1s = [t1pool.tile([128, 512], F32, tag=f"t1sb_{s}_{mb}", name=f"t1sb_{s}_{c}_{mb}")
                           for mb in range(n_t1b)]
                    for mb in range(n_t1b):
                        if mb % 2 == 0:
                            nc.scalar.copy(out=t1s[mb][:], in_=t1b[mb][:])
                        else:
                            nc.vector.tensor_copy(out=t1s[mb][:], in_=t1b[mb][:])
                    ob = pso.tile([128, 512], F32, tag="outps")
                    for ki in range(2):
                        for m in range(M[s]):
                            nc.tensor.matmul(
                                ob[:, ki * 256:ki * 256 + 256],
                                lhsT=t1s[m // 2][:, (m % 2) * 256 + ki * 128:
                                                 (m % 2) * 256 + ki * 128 + 128],
                                rhs=wh_sb[s][:, m * 256:(m + 1) * 256],
                                start=(m == 0), stop=(m == M[s] - 1))
                    osb = outpool.tile([128, 512], F32, tag="outsb")
                    nc.vector.tensor_copy(out=osb[:], in_=ob[:])
                    nc.sync.dma_start(
                        out=images_out[s, c].rearrange("(ki p) j -> p ki j", p=128),
                        in_=osb[:].rearrange("p (ki j) -> p ki j", ki=2))
    nc.compile()
    return nc


# ---------------------------------------------------------------- kernel entry
def kernel(full_imgs, center, bbox_size, heights, widths, starts):
    full = np.ascontiguousarray(np.asarray(full_imgs, dtype=np.float32))
    center = np.asarray(center, dtype=np.float32)
    bbox = np.asarray(bbox_size, dtype=np.float32)
    H = np.asarray(heights).astype(np.int64)
    W = np.asarray(widths).astype(np.int64)
    S = np.asarray(starts).astype(np.int64)
    B = center.shape[0]

    geoms = [image_geometry(center[b], bbox[b], int(H[b]), int(W[b]))
             for b in range(B)]
    # slot assignment: sort by crop area desc; slot0 = top 8, pair k with 15-k
    areas = [g["ch"] * g["cw"] for g in geoms]
    order = list(np.argsort(-np.asarray(areas)))
    pairs = [(int(order[k]), int(order[B - 1 - k])) for k in range(NCORES)]

    def r128(x):
        return ((x + 127) // 128) * 128
    R = [min(r128(max(geoms[p[s]]["ch"] for p in pairs)), 1280) for s in range(2)]
    L = [min(r128(max(geoms[p[s]]["cw"] for p in pairs)), PITCH) for s in range(2)]
    # Hs[s]: per-slot packed buffer size (3*h*w max) + L tail pad
    Hs = [int(max(3 * int(H[p[s]]) * int(W[p[s]]) for p in pairs)) + L[s]
          for s in range(2)]
    Tmax = max(R[s] // 128 for s in range(2))

    lin = (np.arange(CROP, dtype=np.float32) / np.float32(CROP - 1)).astype(np.float32)
    ones = np.ones(CROP, np.float32)
    flhs = np.stack([ones, lin]).astype(np.float32)  # [2(K), 256]: k0=ones, k1=lin

    # host-side tf/hd (3x3 metadata)
    s_ = bbox.astype(np.float32)
    tx = (center[:, 0] - s_ * np.float32(0.5)).astype(np.float32)
    ty = (center[:, 1] - s_ * np.float32(0.5)).astype(np.float32)
    tf = np.zeros((B, 3, 3), np.float32)
    hd = np.zeros((B, 3, 3), np.float32)
    tf[:, 0, 0] = s_; tf[:, 1, 1] = s_; tf[:, 2, 2] = 1.0
    tf[:, 0, 2] = tx; tf[:, 1, 2] = ty
    scale = (np.float32(2.0) * np.float32(CROP - 1) / s_).astype(np.float32)
    hd[:, 0, 0] = scale; hd[:, 1, 1] = scale; hd[:, 2, 2] = 1.0
    hd[:, 0, 2] = (-tx * scale - np.float32(1.0)).astype(np.float32)
    hd[:, 1, 2] = (-ty * scale - np.float32(1.0)).astype(np.float32)

    in_maps = []
    for k in range(NCORES):
        im = {}
        meta_np = np.zeros((128, 2 * 3 * Tmax), np.int32)
        frhs_np = np.zeros((2, 2, 512), np.float32)
        tfhd_np = np.zeros((1, 36), np.float32)
        for s in range(2):
            b = pairs[k][s]
            g = geoms[b]
            h, w = int(H[b]), int(W[b])
            # packed sub-buffer (sharding hint layout), zero tail pad
            buf = np.zeros(Hs[s], np.float32)
            buf[:3 * h * w] = full[S[b]:S[b] + 3 * h * w]
            im[f"img{s}"] = buf
            # clamped read origins; weights are built relative to them
            yread = max(0, min(g["ylo"], h - R[s]))
            xread = max(0, min(g["xlo"], w - 2))
            im[f"wv{s}"] = build_w(g["ys"], g["y0"], g["vy"], yread, R[s])
            im[f"wh{s}"] = build_w(g["xs"], g["x0"], g["vx"], xread, L[s])
            rows = np.arange(128)
            for c in range(3):
                for t in range(R[s] // 128):
                    rr = np.minimum(yread + 128 * t + rows, h - 1)
                    meta_np[:, (s * 3 + c) * Tmax + t] = \
                        c * h * w + rr * w + xread
            frhs_np[0, s, 0::2] = g["xs"]
            frhs_np[0, s, 1::2] = g["ty"]
            frhs_np[1, s, 1::2] = g["s"]
            tfhd_np[0, s * 18:s * 18 + 9] = tf[b].reshape(-1)
            tfhd_np[0, s * 18 + 9:s * 18 + 18] = hd[b].reshape(-1)
        im["meta"] = meta_np
        im["frhs"] = frhs_np.reshape(2, 1024)
        im["flhs"] = flhs.reshape(2, 256)
        im["tfhd"] = tfhd_np
        in_maps.append(im)

    nc = build_program(R, L, Hs)
    res = bass_utils.run_bass_kernel_spmd(
        nc, in_maps, core_ids=list(range(NCORES)), **RUN_KWARGS)
    kernel.last_result = res
    results = res.results

    images = np.zeros((B, 3, CROP, CROP), np.float32)
    flat = np.zeros((B, CROP * CROP, 2), np.float32)
    for k in range(NCORES):
        for s in range(2):
            b = pairs[k][s]
            images[b] = results[k]["images_out"][s]
            flat[b] = results[k]["flat_out"][s]
            tf[b] = results[k]["tfhd_out"][0, s * 18:s * 18 + 9].reshape(3, 3)
            hd[b] = results[k]["tfhd_out"][0, s * 18 + 9:s * 18 + 18].reshape(3, 3)
    return images, flat, tf, hd
